# revision 14
# baseline (speedup 1.0000x reference)
"""Trainium2 Bass kernel for nn_ContrastLoss (supervised-contrastive loss).

Reference computation (B=1024, D=128, C=100, K=32768, N=B+K=33792):
    l   = concat(labels, queue_label.T)          # [N, C]
    w   = labels @ l.T                           # [B, N] shared-class counts
    sim = query @ concat(keys, queue.T).T / T    # [B, N]
    logits = sim - rowmax(sim)
    denom  = sum(exp(logits) * logits_mask, 1)   # logits_mask zeros keys-diag
    loss = -(T/BT) * sqrt(w/max(w)) * (logits - log(denom))

Restructurings:
  * max(w) == max_i rowsum(labels_i) exactly (binary labels, diag of the
    keys block included), computed on-device from the labels.T block of
    rhs_w via a ones-vector matmul + free-dim max + broadcast matmul.
  * Constant softmax stabilizer m=1.0 (inputs are L2-normalized so raw
    sim is in [-1,1]); the shift cancels in log_prob exactly.
  * qT is host-scaled by -1/T so the sim matmul yields r = -raw/T, and
    loss = s * (r + lnb) with lnb = ln(denom') + m/T and
    s = sqrt(w*(T/BT)^2/wmax) >= 0 folding the sign and the w>0 mask.
    The whole elementwise tail is ONE scalar_tensor_tensor per chunk.

Sharding: pure data-parallel over the B (query row) dimension -- core c
owns rows [c*128, (c+1)*128) and ALL N=33792 similarity columns: NO
collectives, no cross-core dependencies, so cores are immune to launch
stagger (the old TP baseline lost ~70us/core to a start barrier + an
AllGather, much more under staggered launches).

Per-core schedule (engine totals ~ TE 64us, ACT 72us, DVE 81us):
  - sim matmul (bf16) streams rhs_sim in 2048-col piece tiles from two
    DMA queues; each 1024-col psum chunk is copied to a resident bf16
    r = -raw/T buffer by DVE (cheaper than re-running the matmul pass).
  - ACT runs all Exps first (4096-col chunks over r, accum_out row
    sums), so lnb is ready early; then all Sqrts (w matmul, fp8 exact
    for 0/1 labels) into 33 resident bf16 s tiles.
  - After lnb: 33 STTs (r + lnb)*s -> staged f32 output, DMAed on both
    queues; the final group is split across queues to shorten the tail.
"""

import numpy as np
import ml_dtypes

import concourse.bass as bass
import concourse.mybir as mybir
import concourse.tile as tile
from concourse import bacc
from concourse.bass_utils import run_bass_kernel_spmd

F32 = mybir.dt.float32
BF16 = mybir.dt.bfloat16
F8 = mybir.dt.float8e4
ALU = mybir.AluOpType
ACTF = mybir.ActivationFunctionType
AXX = mybir.AxisListType.X

B, D, C, KQ = 1024, 128, 100, 32768
NCORES = 8
RPC = B // NCORES          # 128 query rows per core
N = B + KQ                 # 33792 similarity columns, all on every core
CH = 1024                  # column chunk (psum tile: 2 banks)
NCH = N // CH              # 33
GRP = 3                    # chunks per output-DMA group
NGRP = NCH // GRP          # 11
SP = 2048                  # rhs_sim DMA piece (cols)
RP = 4096                  # raw piece (cols)
WP = 4096                  # rhs_w DMA piece (cols)
STAB = 1.0                 # constant softmax stabilizer


def _pieces(total, size):
    return [(a, min(a + size, total)) for a in range(0, total, size)]


def _build_nc(Tf: float, BTf: float):
    nc = bacc.Bacc("TRN2", target_bir_lowering=False, debug=False,
                   num_devices=NCORES)

    qT_d = nc.dram_tensor("qT", [D, RPC], BF16, kind="ExternalInput")
    rhs_sim_d = nc.dram_tensor("rhs_sim", [D, N], BF16, kind="ExternalInput")
    labT_d = nc.dram_tensor("labT", [C, RPC], F8, kind="ExternalInput")
    rhs_w_d = nc.dram_tensor("rhs_w", [C, N], F8, kind="ExternalInput")
    dmask_d = nc.dram_tensor("dmask", [RPC, B], BF16, kind="ExternalInput")
    out_d = nc.dram_tensor("out", [RPC, N], F32, kind="ExternalOutput")

    sim_pieces = _pieces(N, SP)     # 17: 16x2048 + 1024
    raw_pieces = _pieces(N, RP)     # 9: 8x4096 + 1024
    w_pieces = _pieces(N, WP)       # 9

    with tile.TileContext(nc) as tc:
        with (
            tc.tile_pool(name="const", bufs=1) as const,
            tc.tile_pool(name="simp", bufs=4) as simp,
            tc.tile_pool(name="spool", bufs=3) as spool,
            tc.tile_pool(name="stg", bufs=2) as stg,
            tc.tile_pool(name="psA", bufs=2, space="PSUM") as psA,
            tc.tile_pool(name="psB", bufs=2, space="PSUM") as psB,
        ):
            # ---- input DMAs, alternating between the two HW queues --------
            qTc = const.tile([D, RPC], BF16)
            nc.sync.dma_start(out=qTc[:], in_=qT_d[:])
            dmask = const.tile([RPC, B], BF16)
            nc.sync.dma_start(out=dmask[:], in_=dmask_d[:])
            labTc = const.tile([C, RPC], F8)
            nc.gpsimd.dma_start(out=labTc[:], in_=labT_d[:])

            # rhs_w head first (wmax needs cols 0:B early), rest later below
            wt = []
            for pi, (a, b) in enumerate(w_pieces):
                t = const.tile([C, b - a], F8, name=f"wt{pi}")
                wt.append(t)
            nc.gpsimd.dma_start(out=wt[0][:], in_=rhs_w_d[:, 0:WP])

            st_tiles = []
            for pi, (a, b) in enumerate(sim_pieces):
                t = simp.tile([D, b - a], BF16, tag="sp", name=f"sp{pi}")
                eng = nc.sync if pi % 2 == 0 else nc.gpsimd
                eng.dma_start(out=t[:], in_=rhs_sim_d[:, a:b])
                st_tiles.append(t)

            # rest of rhs_w (needed from ~mid-kernel only)
            for pi, (a, b) in enumerate(w_pieces):
                if pi == 0:
                    continue
                nc.gpsimd.dma_start(out=wt[pi][:], in_=rhs_w_d[:, a:b])

            ebias = const.tile([RPC, 1], F32)
            nc.vector.memset(ebias, -STAB / Tf)

            # ---- sim matmuls + DVE copy psum -> resident bf16 r = -raw/T --
            raw = []
            for pi, (a, b) in enumerate(raw_pieces):
                raw.append(const.tile([D, b - a], BF16, name=f"raw{pi}"))
            for k in range(NCH):
                base = k * CH
                ps = psA.tile([RPC, CH], F32, tag="ps")
                for h in range(2):
                    a = base + h * 512
                    sp = a // SP
                    off = a - sim_pieces[sp][0]
                    nc.tensor.matmul(ps[:, h * 512:(h + 1) * 512], qTc[:],
                                     st_tiles[sp][:, off:off + 512],
                                     start=True, stop=True)
                rp = k // 4
                ro = base - raw_pieces[rp][0]
                nc.vector.tensor_scalar_add(raw[rp][:, ro:ro + CH], ps[:], 0.0)

            # ---- all Exps (big chunks over r), row sums via accum_out -----
            e_keys = const.tile([RPC, B], F32)
            e_scr = const.tile([RPC, RP], BF16)
            acc = const.tile([RPC, len(raw_pieces) + 1], F32)
            nc.scalar.activation(e_keys[:], raw[0][:, 0:B], ACTF.Exp,
                                 bias=ebias[:], scale=-1.0,
                                 accum_out=acc[:, 0:1])
            nc.scalar.activation(e_scr[:, 0:RP - B], raw[0][:, B:RP], ACTF.Exp,
                                 bias=ebias[:], scale=-1.0,
                                 accum_out=acc[:, 1:2])
            for pi in range(1, len(raw_pieces)):
                a, b = raw_pieces[pi]
                nc.scalar.activation(e_scr[:, 0:b - a], raw[pi][:], ACTF.Exp,
                                     bias=ebias[:], scale=-1.0,
                                     accum_out=acc[:, pi + 1:pi + 2])

            # ---- denominator: subtract self-diagonal, take ln -------------
            nc.vector.tensor_mul(e_keys[:], e_keys[:], dmask[:])
            corr = const.tile([RPC, 1], F32)
            nc.vector.tensor_reduce(corr[:], e_keys[:], axis=AXX, op=ALU.add)
            dn = const.tile([RPC, 1], F32)
            nc.vector.tensor_reduce(dn[:], acc[:], axis=AXX, op=ALU.add)
            dn2 = const.tile([RPC, 1], F32)
            nc.vector.tensor_sub(dn2[:], dn[:], corr[:])
            lnd = const.tile([RPC, 1], F32)
            nc.scalar.activation(lnd[:], dn2[:], ACTF.Ln)
            lnb = const.tile([RPC, 1], F32)
            nc.vector.tensor_scalar_add(lnb[:], lnd[:], STAB / Tf)

            # ---- wmax = max_i rowsum(labels_i), via two tiny matmuls ------
            ones_c = const.tile([C, 1], F8)
            nc.vector.memset(ones_c, 1.0)
            ones_r = const.tile([1, RPC], F32)
            nc.vector.memset(ones_r, 1.0)
            pm = psA.tile([RPC, CH], F32, tag="ps")
            nc.tensor.matmul(pm[0:1, 0:512], ones_c[:], wt[0][:, 0:512],
                             start=True, stop=True)
            nc.tensor.matmul(pm[0:1, 512:B], ones_c[:], wt[0][:, 512:B],
                             start=True, stop=True)
            wm = const.tile([1, 1], F32)
            nc.vector.tensor_reduce(wm[:], pm[0:1, 0:B], axis=AXX, op=ALU.max)
            pb = psA.tile([RPC, CH], F32, tag="ps")
            nc.tensor.matmul(pb[:, 0:1], ones_r[:], wm[:],
                             start=True, stop=True)
            winv = const.tile([RPC, 1], F32)
            nc.vector.reciprocal(winv[:], pb[:, 0:1])
            sq_scale = const.tile([RPC, 1], F32)
            nc.vector.tensor_scalar_mul(sq_scale[:], winv[:], (Tf / BTf) ** 2)

            # ---- phase B: w matmul, s = sqrt(w*c), out = (r + lnb) * s ----
            for k in range(NCH):
                base = k * CH
                g, j = divmod(k, GRP)
                if j == 0:
                    st = stg.tile([RPC, GRP * CH], F32, tag="st")
                ps_w = psB.tile([RPC, CH], F32, tag="pw")
                for h in range(2):
                    a = base + h * 512
                    wp = a // WP
                    off = a - w_pieces[wp][0]
                    nc.tensor.matmul(ps_w[:, h * 512:(h + 1) * 512], labTc[:],
                                     wt[wp][:, off:off + 512],
                                     start=True, stop=True)
                s = spool.tile([RPC, CH], F32, tag="s")
                nc.scalar.activation(s[:], ps_w[:], ACTF.Sqrt,
                                     scale=sq_scale[:])
                rp = k // 4
                ro = base - raw_pieces[rp][0]
                nc.vector.scalar_tensor_tensor(
                    st[:, j * CH:(j + 1) * CH], raw[rp][:, ro:ro + CH],
                    lnb[:], s[:], op0=ALU.add, op1=ALU.mult)
                if j == GRP - 1:
                    a = g * GRP * CH
                    if g < NGRP - 1:
                        eng = nc.sync if (g % 2 == 0) else nc.gpsimd
                        eng.dma_start(out=out_d[:, a:a + GRP * CH], in_=st[:])
                    else:
                        # final group: split across both queues for the tail
                        half = GRP * CH // 2
                        nc.sync.dma_start(out=out_d[:, a:a + half],
                                          in_=st[:, 0:half])
                        nc.gpsimd.dma_start(out=out_d[:, a + half:a + GRP * CH],
                                            in_=st[:, half:GRP * CH])
    nc.compile()
    return nc


def _host_prep(query, keys, labels, queue, queue_label, Tf):
    bf16 = ml_dtypes.bfloat16
    f8 = ml_dtypes.float8_e4m3fn
    query = np.asarray(query, np.float32)
    keys = np.asarray(keys, np.float32)
    labels = np.asarray(labels, np.float32)
    queue = np.asarray(queue, np.float32)
    queue_label = np.asarray(queue_label, np.float32)

    # Pre-scaled by -1/T: the sim matmul then produces r = -raw/T directly,
    # letting the output stage fuse (lnb - raw/T)*s into one STT.
    qT = np.ascontiguousarray((query.T * (-1.0 / Tf)).astype(bf16))
    rhs_sim = np.concatenate([keys.T, queue], axis=1).astype(bf16)
    labT = np.ascontiguousarray(labels.T.astype(f8))          # [C, B] exact
    rhs_w = np.ascontiguousarray(
        np.concatenate([labels.T, queue_label], axis=1).astype(f8))

    in_maps = []
    idx = np.arange(RPC)
    for c in range(NCORES):
        rows = slice(c * RPC, (c + 1) * RPC)
        dmask = np.zeros((RPC, B), np.float32)
        dmask[idx, c * RPC + idx] = 1.0
        in_maps.append({
            "qT": np.ascontiguousarray(qT[:, rows]),
            "rhs_sim": rhs_sim,
            "labT": np.ascontiguousarray(labT[:, rows]),
            "rhs_w": rhs_w,
            "dmask": dmask.astype(bf16),
        })
    return in_maps


def _gather_output(results):
    return np.concatenate([results[c]["out"] for c in range(NCORES)], axis=0)


def kernel(query, keys, labels, queue, queue_label, K, T, BT, **_unused):
    Tf = float(np.asarray(T))
    BTf = float(np.asarray(BT))
    nc = _build_nc(Tf, BTf)
    in_maps = _host_prep(query, keys, labels, queue, queue_label, Tf)
    res = run_bass_kernel_spmd(nc, in_maps, list(range(NCORES)))
    return _gather_output(res.results)


# Re-usable entry for test.py: returns (output, BassKernelResults) so the
# harness there can pull exec_time_ns / profile out of a traced run.
def kernel_traced(query, keys, labels, queue, queue_label, K, T, BT,
                  trace=False, **run_kwargs):
    Tf = float(np.asarray(T))
    BTf = float(np.asarray(BT))
    nc = _build_nc(Tf, BTf)
    in_maps = _host_prep(query, keys, labels, queue, queue_label, Tf)
    res = run_bass_kernel_spmd(nc, in_maps, list(range(NCORES)),
                               trace=trace, **run_kwargs)
    return _gather_output(res.results), res


# revision 18
# speedup vs baseline: 1.0565x; 1.0565x over previous
"""Trainium2 Bass kernel for nn_ContrastLoss (supervised-contrastive loss).

Reference computation (B=1024, D=128, C=100, K=32768, N=B+K=33792):
    l   = concat(labels, queue_label.T)          # [N, C]
    w   = labels @ l.T                           # [B, N] shared-class counts
    sim = query @ concat(keys, queue.T).T / T    # [B, N]
    logits = sim - rowmax(sim)
    denom  = sum(exp(logits) * logits_mask, 1)   # logits_mask zeros keys-diag
    loss = -(T/BT) * sqrt(w/max(w)) * (logits - log(denom))

Restructurings:
  * max(w) == max_i rowsum(labels_i) exactly (binary labels, diag of the
    keys block included), computed on-device from the labels.T block of
    rhs_w via a ones-vector matmul + free-dim max + broadcast matmul.
  * Constant softmax stabilizer m=1.0 (inputs are L2-normalized so raw
    sim is in [-1,1]); the shift cancels in log_prob exactly.
  * qT is host-scaled by -1/T so the sim matmul yields r = -raw/T, and
    loss = s * (r + lnb) with lnb = ln(denom') + m/T and
    s = sqrt(w*(T/BT)^2/wmax) >= 0 folding the sign and the w>0 mask.
    The whole elementwise tail is ONE scalar_tensor_tensor per chunk.

Sharding: pure data-parallel over the B (query row) dimension -- core c
owns rows [c*128, (c+1)*128) and ALL N=33792 similarity columns: NO
collectives, no cross-core dependencies, so cores are immune to launch
stagger (the old TP baseline lost ~70us/core to a start barrier + an
AllGather, much more under staggered launches).

Per-core schedule (engine totals ~ TE 64us, ACT 72us, DVE 81us):
  - sim matmul (bf16) streams rhs_sim in 2048-col piece tiles from two
    DMA queues; each 1024-col psum chunk is copied to a resident bf16
    r = -raw/T buffer by DVE (cheaper than re-running the matmul pass).
  - ACT runs all Exps first (4096-col chunks over r, accum_out row
    sums), so lnb is ready early; then all Sqrts (w matmul, fp8 exact
    for 0/1 labels) into 33 resident bf16 s tiles.
  - After lnb: 33 STTs (r + lnb)*s -> staged f32 output, DMAed on both
    queues; the final group is split across queues to shorten the tail.
"""

import numpy as np
import ml_dtypes

import concourse.bass as bass
import concourse.mybir as mybir
import concourse.tile as tile
from concourse import bacc
from concourse.bass_utils import run_bass_kernel_spmd

F32 = mybir.dt.float32
BF16 = mybir.dt.bfloat16
F8 = mybir.dt.float8e4
ALU = mybir.AluOpType
ACTF = mybir.ActivationFunctionType
AXX = mybir.AxisListType.X

B, D, C, KQ = 1024, 128, 100, 32768
NCORES = 8
RPC = B // NCORES          # 128 query rows per core
N = B + KQ                 # 33792 similarity columns, all on every core
CH = 1024                  # column chunk (psum tile: 2 banks)
NCH = N // CH              # 33
GRP = 3                    # chunks per output-DMA group
NGRP = NCH // GRP          # 11
SP = 2048                  # rhs_sim DMA piece (cols)
RP = 4096                  # raw piece (cols)
WP = 4096                  # rhs_w DMA piece (cols)
STAB = 1.0                 # constant softmax stabilizer


def _pieces(total, size):
    return [(a, min(a + size, total)) for a in range(0, total, size)]


def _build_nc(Tf: float, BTf: float):
    nc = bacc.Bacc("TRN2", target_bir_lowering=False, debug=False,
                   num_devices=NCORES)

    qT_d = nc.dram_tensor("qT", [D, RPC], BF16, kind="ExternalInput")
    rhs_sim_d = nc.dram_tensor("rhs_sim", [D, N], BF16, kind="ExternalInput")
    labT_d = nc.dram_tensor("labT", [C, RPC], F8, kind="ExternalInput")
    rhs_w_d = nc.dram_tensor("rhs_w", [C, N], F8, kind="ExternalInput")
    dmask_d = nc.dram_tensor("dmask", [RPC, B], BF16, kind="ExternalInput")
    out_d = nc.dram_tensor("out", [RPC, N], F32, kind="ExternalOutput")

    sim_pieces = _pieces(N, SP)     # 17: 16x2048 + 1024
    raw_pieces = _pieces(N, RP)     # 9: 8x4096 + 1024
    w_pieces = _pieces(N, WP)       # 9

    with tile.TileContext(nc) as tc:
        with (
            tc.tile_pool(name="const", bufs=1) as const,
            tc.tile_pool(name="simp", bufs=4) as simp,
            tc.tile_pool(name="spool", bufs=3) as spool,
            tc.tile_pool(name="stg", bufs=2) as stg,
            tc.tile_pool(name="psA", bufs=2, space="PSUM") as psA,
            tc.tile_pool(name="psB", bufs=2, space="PSUM") as psB,
        ):
            # ---- input DMAs, alternating between the two HW queues --------
            qTc = const.tile([D, RPC], BF16)
            nc.sync.dma_start(out=qTc[:], in_=qT_d[:])
            labTc = const.tile([C, RPC], F8)
            nc.gpsimd.dma_start(out=labTc[:], in_=labT_d[:])

            # rhs_w head first (wmax needs cols 0:B early), rest later below
            wt = []
            for pi, (a, b) in enumerate(w_pieces):
                t = const.tile([C, b - a], F8, name=f"wt{pi}")
                wt.append(t)
            nc.gpsimd.dma_start(out=wt[0][:], in_=rhs_w_d[:, 0:WP])

            st_tiles = []
            for pi, (a, b) in enumerate(sim_pieces):
                t = simp.tile([D, b - a], BF16, tag="sp", name=f"sp{pi}")
                eng = nc.sync if pi % 2 == 0 else nc.gpsimd
                eng.dma_start(out=t[:], in_=rhs_sim_d[:, a:b])
                st_tiles.append(t)

            # rest of rhs_w (needed from ~mid-kernel only); dmask needed ~45us
            for pi, (a, b) in enumerate(w_pieces):
                if pi == 0:
                    continue
                nc.gpsimd.dma_start(out=wt[pi][:], in_=rhs_w_d[:, a:b])
            dmask = const.tile([RPC, B], BF16)
            nc.sync.dma_start(out=dmask[:], in_=dmask_d[:])

            ebias = const.tile([RPC, 1], F32)
            nc.vector.memset(ebias, -STAB / Tf)

            # ---- sim matmuls + DVE copy psum -> resident bf16 r = -raw/T --
            raw = []
            for pi, (a, b) in enumerate(raw_pieces):
                raw.append(const.tile([D, b - a], BF16, name=f"raw{pi}"))
            for k in range(NCH):
                base = k * CH
                ps = psA.tile([RPC, CH], F32, tag="ps")
                for h in range(2):
                    a = base + h * 512
                    sp = a // SP
                    off = a - sim_pieces[sp][0]
                    nc.tensor.matmul(ps[:, h * 512:(h + 1) * 512], qTc[:],
                                     st_tiles[sp][:, off:off + 512],
                                     start=True, stop=True)
                rp = k // 4
                ro = base - raw_pieces[rp][0]
                nc.vector.tensor_scalar_add(raw[rp][:, ro:ro + CH], ps[:], 0.0)

            # ---- all Exps (big chunks over r), row sums via accum_out -----
            e_keys = const.tile([RPC, B], F32)
            e_scr = const.tile([RPC, RP], BF16)
            acc = const.tile([RPC, len(raw_pieces) + 1], F32)
            nc.scalar.activation(e_keys[:], raw[0][:, 0:B], ACTF.Exp,
                                 bias=ebias[:], scale=-1.0,
                                 accum_out=acc[:, 0:1])
            nc.scalar.activation(e_scr[:, 0:RP - B], raw[0][:, B:RP], ACTF.Exp,
                                 bias=ebias[:], scale=-1.0,
                                 accum_out=acc[:, 1:2])
            for pi in range(1, len(raw_pieces)):
                a, b = raw_pieces[pi]
                nc.scalar.activation(e_scr[:, 0:b - a], raw[pi][:], ACTF.Exp,
                                     bias=ebias[:], scale=-1.0,
                                     accum_out=acc[:, pi + 1:pi + 2])

            # ---- denominator: subtract self-diagonal, take ln -------------
            nc.vector.tensor_mul(e_keys[:], e_keys[:], dmask[:])
            corr = const.tile([RPC, 1], F32)
            nc.vector.tensor_reduce(corr[:], e_keys[:], axis=AXX, op=ALU.add)
            dn = const.tile([RPC, 1], F32)
            nc.vector.tensor_reduce(dn[:], acc[:], axis=AXX, op=ALU.add)
            dn2 = const.tile([RPC, 1], F32)
            nc.vector.tensor_sub(dn2[:], dn[:], corr[:])
            lnd = const.tile([RPC, 1], F32)
            nc.scalar.activation(lnd[:], dn2[:], ACTF.Ln)
            lnb = const.tile([RPC, 1], F32)
            nc.vector.tensor_scalar_add(lnb[:], lnd[:], STAB / Tf)

            # ---- wmax = max_i rowsum(labels_i), via two tiny matmuls ------
            ones_c = const.tile([C, 1], F8)
            nc.vector.memset(ones_c, 1.0)
            ones_r = const.tile([1, RPC], F32)
            nc.vector.memset(ones_r, 1.0)
            pm = psA.tile([RPC, CH], F32, tag="ps")
            nc.tensor.matmul(pm[0:1, 0:512], ones_c[:], wt[0][:, 0:512],
                             start=True, stop=True)
            nc.tensor.matmul(pm[0:1, 512:B], ones_c[:], wt[0][:, 512:B],
                             start=True, stop=True)
            wm = const.tile([1, 1], F32)
            nc.vector.tensor_reduce(wm[:], pm[0:1, 0:B], axis=AXX, op=ALU.max)
            pb = psA.tile([RPC, CH], F32, tag="ps")
            nc.tensor.matmul(pb[:, 0:1], ones_r[:], wm[:],
                             start=True, stop=True)
            winv = const.tile([RPC, 1], F32)
            nc.vector.reciprocal(winv[:], pb[:, 0:1])
            sq_scale = const.tile([RPC, 1], F32)
            nc.vector.tensor_scalar_mul(sq_scale[:], winv[:], (Tf / BTf) ** 2)

            # ---- phase B: w matmul, s = sqrt(w*c), out = (r + lnb) * s ----
            # The wait hint keeps the static scheduler from hoisting phase-B
            # matmuls into the sim-matmul window: the sim chain feeds lnb,
            # which gates every STT, so any TE time stolen before lnb is a
            # direct hit to the critical path.
            ctx_b = tc.tile_wait_until(0.045)
            ctx_b.__enter__()
            for k in range(NCH):
                base = k * CH
                g, j = divmod(k, GRP)
                if j == 0:
                    st = stg.tile([RPC, GRP * CH], F32, tag="st")
                ps_w = psB.tile([RPC, CH], F32, tag="pw")
                for h in range(2):
                    a = base + h * 512
                    wp = a // WP
                    off = a - w_pieces[wp][0]
                    nc.tensor.matmul(ps_w[:, h * 512:(h + 1) * 512], labTc[:],
                                     wt[wp][:, off:off + 512],
                                     start=True, stop=True)
                s = spool.tile([RPC, CH], F32, tag="s")
                nc.scalar.activation(s[:], ps_w[:], ACTF.Sqrt,
                                     scale=sq_scale[:])
                rp = k // 4
                ro = base - raw_pieces[rp][0]
                nc.vector.scalar_tensor_tensor(
                    st[:, j * CH:(j + 1) * CH], raw[rp][:, ro:ro + CH],
                    lnb[:], s[:], op0=ALU.add, op1=ALU.mult)
                if j == GRP - 1:
                    a = g * GRP * CH
                    if g < NGRP - 1:
                        eng = nc.sync if (g % 2 == 0) else nc.gpsimd
                        eng.dma_start(out=out_d[:, a:a + GRP * CH], in_=st[:])
                    else:
                        # final group: split across both queues for the tail
                        half = GRP * CH // 2
                        nc.sync.dma_start(out=out_d[:, a:a + half],
                                          in_=st[:, 0:half])
                        nc.gpsimd.dma_start(out=out_d[:, a + half:a + GRP * CH],
                                            in_=st[:, half:GRP * CH])
            ctx_b.__exit__(None, None, None)
    nc.compile()
    return nc


def _host_prep(query, keys, labels, queue, queue_label, Tf):
    bf16 = ml_dtypes.bfloat16
    f8 = ml_dtypes.float8_e4m3fn
    query = np.asarray(query, np.float32)
    keys = np.asarray(keys, np.float32)
    labels = np.asarray(labels, np.float32)
    queue = np.asarray(queue, np.float32)
    queue_label = np.asarray(queue_label, np.float32)

    # Pre-scaled by -1/T: the sim matmul then produces r = -raw/T directly,
    # letting the output stage fuse (lnb - raw/T)*s into one STT.
    qT = np.ascontiguousarray((query.T * (-1.0 / Tf)).astype(bf16))
    rhs_sim = np.concatenate([keys.T, queue], axis=1).astype(bf16)
    labT = np.ascontiguousarray(labels.T.astype(f8))          # [C, B] exact
    rhs_w = np.ascontiguousarray(
        np.concatenate([labels.T, queue_label], axis=1).astype(f8))

    in_maps = []
    idx = np.arange(RPC)
    for c in range(NCORES):
        rows = slice(c * RPC, (c + 1) * RPC)
        dmask = np.zeros((RPC, B), np.float32)
        dmask[idx, c * RPC + idx] = 1.0
        in_maps.append({
            "qT": np.ascontiguousarray(qT[:, rows]),
            "rhs_sim": rhs_sim,
            "labT": np.ascontiguousarray(labT[:, rows]),
            "rhs_w": rhs_w,
            "dmask": dmask.astype(bf16),
        })
    return in_maps


def _gather_output(results):
    return np.concatenate([results[c]["out"] for c in range(NCORES)], axis=0)


def kernel(query, keys, labels, queue, queue_label, K, T, BT, **_unused):
    Tf = float(np.asarray(T))
    BTf = float(np.asarray(BT))
    nc = _build_nc(Tf, BTf)
    in_maps = _host_prep(query, keys, labels, queue, queue_label, Tf)
    res = run_bass_kernel_spmd(nc, in_maps, list(range(NCORES)))
    return _gather_output(res.results)


# Re-usable entry for test.py: returns (output, BassKernelResults) so the
# harness there can pull exec_time_ns / profile out of a traced run.
def kernel_traced(query, keys, labels, queue, queue_label, K, T, BT,
                  trace=False, **run_kwargs):
    Tf = float(np.asarray(T))
    BTf = float(np.asarray(BT))
    nc = _build_nc(Tf, BTf)
    in_maps = _host_prep(query, keys, labels, queue, queue_label, Tf)
    res = run_bass_kernel_spmd(nc, in_maps, list(range(NCORES)),
                               trace=trace, **run_kwargs)
    return _gather_output(res.results), res
